# revision 27
# baseline (speedup 1.0000x reference)
"""Trainium2 Bass kernel for nn_CustomLSTM (B=256, T=1024, I=64, H=256, O=8).

Strategy: data-parallel over batch across 8 cores (32 batch rows each).
Per core, the LSTM recurrence runs with everything in feature-on-partition
("transposed") layout:
  - gate pre-activations accumulate in a PSUM ring, one slot per timestep;
    x-projections (+ bias, via an appended ones-row on x) are matmul'd in
    ahead of time so the serial path only streams the Wh tiles
  - C=2 batch-chains run offset by one timestep in a ladder; each chain's
    per-step serial path is PE(16 mm) -> ACT(one tanh over all 8 gate
    tiles, via sigmoid(g)=(tanh(g/2)+1)/2 with the 1/2's folded into the
    weights) -> DVE(2 fused scalar_tensor_tensor ops) -> ACT(tanh c) ->
    DVE(h update) -> PE
  - the state tile X = [2c | tanh(gates)] with gate slot order
    [chat,f,i,o] makes the cell update 2c' = (tf+1)*2c*0.5 + (ti+1)*chat
    two DVE ops: u = (X[4:8]+1)*X[0:4]; X[0:2] = 0.5*u[0:2] + u[2:4].
    Storing c as 2c removes the c-halving op from the loop entirely.
Output projection h_T @ W_ho runs on-device; b_ho is added on the host.

This file is self-contained: shapes/sharding are hardcoded to the problem.
"""

import sys

sys.path.insert(0, "/opt/trn_rl_repo")

import numpy as np

import concourse.bass as bass
import concourse.mybir as mybir
from concourse.tile import TileContext
from concourse.vector_clock import ScopedClock, VectorClock

# ----------------------------------------------------------------------------
# Problem constants (full problem, then per-core)
# ----------------------------------------------------------------------------
B_FULL, T, I, H, O = 256, 1024, 64, 256, 8
NCORES = 8
B = B_FULL // NCORES          # 32 batch rows per core
G = 4 * H                     # 1024 gate pre-activations
KT = H // 128                 # 2 k-tiles for the h-part
MT = G // 128                 # 8 m-tiles of gate columns

# Tunables
C = 2  # batch chains (ladder depth)
Bc = B // C                   # batch per chain
# S=1: one step's x-projection per pass (keeps PE bursts small and off
# the serial h-path).
S = 1                         # x-projection lookahead (steps per x-phase)
R = S + 2                     # PSUM ring slots; R + C <= 8
SC = 64                       # x DMA superchunk (steps per DMA)
import os as _os
# KERNEL_DT: "bf16" (default; both paths bf16, W_ho/psum/c stay fp32),
#            "mixed" (x path fp32), "fp32" (both)
_mode = _os.environ.get("KERNEL_DT", "bf16")
HDT = mybir.dt.float32 if _mode == "fp32" else mybir.dt.bfloat16
XDT = mybir.dt.bfloat16 if _mode == "bf16" else mybir.dt.float32
SIG_SPLIT = False             # True: sig(f,i) + sig(o) separate; False: one sig op
M_OUTER = False               # True: loop m outer / k inner in the h-phase
# TANH_TRICK: sigmoid(g)=(tanh(g/2)+1)/2 with the 1/2's pre-folded into the
# weights: ONE tanh ACT op covers all 8 gate tiles. h-state is stored as
# 2h (Wh, W_ho pre-halved); c kept true via an off-critical-path halving.
TANH_TRICK = True
# XBF: keep the nonlinear-phase state tile X (2c + tanh outputs), u and th
# in bf16 -- enables DVE 2x perf mode; c-recurrence is contractive so the
# bf16 c-state costs ~2e-3 rel err over 1024 steps (verified vs numpy).
XBF = _os.environ.get("KERNEL_XBF", "0") == "1"


# m-tile permutation of gate columns: packed slot order [c0 c1 f0 f1 i0 i1 o0 o1]
# chosen so the state tile X = [2c(2) | tanh-out(8)] gives ADJACENT operands:
#   u = (X[4:8]{f,i} + 1) * X[0:4]{2c, chat}   -- one stt covers u1|u2
# reference gate column order is [f(0:256) i(256:512) c(512:768) o(768:1024)]
M_PERM = [4, 5, 0, 1, 2, 3, 6, 7]  # source m-tile index for each packed slot


# ----------------------------------------------------------------------------
# Tile walrus workaround: this container's walrus accepts at most ONE sync
# wait per instruction.  (a) patch the TileContext tail drain to spread its
# waits over per-proc SP nops; (b) after build, hoist excess waits from any
# instruction onto same-engine nops placed immediately before it.
# ----------------------------------------------------------------------------
def _patched_drain_and_barrier(self, tick_clock, wait_clock):
    nc = self.nc
    g = tick_clock.global_clock
    n = len(g)
    for p in range(n):
        if g[p] == 0:
            continue
        vc = VectorClock([g[q] if q == p else 0 for q in range(n)])
        nop = nc.sync.nop(nofuse=True)
        wait_clock.add_sem_waits(nop.ins, ScopedClock({None: vc}))
    nc.sync.drain()
    nc.all_engine_barrier()
    assert self.sems is not None
    popped = nc._tile_sem_poison_stack.pop()
    assert popped is self._sem_poison
    nc.clear_and_free_semaphores(list(self.sems.allocated().values()))
    nc.all_engine_barrier()


def apply_tile_patch():
    TileContext._drain_and_barrier = _patched_drain_and_barrier


def legalize_waits(nc, limit=1):
    """Hoist excess sem waits (>limit per instruction) onto same-engine nops
    inserted immediately before the instruction."""
    eng_builders = {
        mybir.EngineType.PE: nc.tensor,
        mybir.EngineType.DVE: nc.vector,
        mybir.EngineType.Activation: nc.scalar,
        mybir.EngineType.Pool: nc.gpsimd,
        mybir.EngineType.SP: nc.sync,
    }
    n_hoisted = 0
    for f in nc.m.functions:
        for bb in f.blocks:
            snapshot = list(bb.instructions)
            fixes = []  # (index, inst, excess_waits)
            for idx, inst in enumerate(snapshot):
                si = inst.sync_info
                waits = list(si.on_wait) if si and si.on_wait else []
                if len(waits) > limit:
                    fixes.append((idx, inst, waits))
            if not fixes:
                continue
            # create nops via the engine builders (they append to cur_bb;
            # pop them back off to place manually)
            out = []
            prev = 0
            for idx, inst, waits in fixes:
                out.extend(snapshot[prev:idx])
                keep = waits[-limit:]
                excess = waits[:-limit]
                for w in excess:
                    builder = eng_builders[inst.engine]
                    nop_bi = builder.nop(nofuse=True)
                    nop_inst = nop_bi.ins
                    # remove from wherever the builder appended it
                    cur = nc.cur_bb.bb
                    assert cur.instructions[-1] is nop_inst
                    cur.instructions.pop()
                    nop_inst.sync_info = mybir.SyncInfo(on_wait=[w], on_update=[])
                    out.append(nop_inst)
                    n_hoisted += 1
                inst.sync_info = mybir.SyncInfo(
                    on_wait=keep, on_update=list(inst.sync_info.on_update or [])
                )
                out.append(inst)
                prev = idx + 1
            out.extend(snapshot[prev:])
            bb.instructions = out
    return n_hoisted


# ----------------------------------------------------------------------------
# Kernel build
# ----------------------------------------------------------------------------
ROLES = {}  # inst name -> (role, pass, chain); populated during build for sim


def _tag(bi, role, p, j):
    try:
        ROLES[bi.ins.name] = (role, p, j)
    except Exception:
        pass
    return bi


def build_nc(t_steps=T, hdt=None, xdt=None, dt=None, c=None):
    """Build the per-core Bass program. Returns nc."""
    ROLES.clear()
    C_ = C if c is None else c
    Bc_ = B // C_
    R_ = S + C_
    if dt is not None:
        hdt = xdt = dt
    hdt = HDT if hdt is None else hdt
    xdt = XDT if xdt is None else xdt
    apply_tile_patch()
    fp32 = mybir.dt.float32
    Af = mybir.ActivationFunctionType

    nc = bass.Bass()
    xT_d = nc.dram_tensor("xT", [I + 1, t_steps * B], xdt, kind="ExternalInput")
    Wh_d = nc.dram_tensor("Wh", [128, KT * G], hdt, kind="ExternalInput")
    Wx_d = nc.dram_tensor("Wx", [I + 1, G], xdt, kind="ExternalInput")
    Who_d = nc.dram_tensor("Who", [128, KT * O], fp32, kind="ExternalInput")
    y_d = nc.dram_tensor("y", [B, O], fp32, kind="ExternalOutput")

    n_pass = t_steps + C_ - 1

    with TileContext(nc) as tc:
        with (
            tc.tile_pool(name="wpool", bufs=1) as wpool,
            tc.tile_pool(name="state", bufs=1) as state,
            tc.tile_pool(name="xbuf", bufs=3) as xbuf,
            tc.tile_pool(name="gbuf", bufs=2 * C_ + 1) as gbuf,
            tc.tile_pool(name="tbuf", bufs=2 * C_ + 2) as tbuf,
            tc.tile_pool(name="ring", bufs=1, space="PSUM") as ringp,
            tc.tile_pool(name="ypsum", bufs=1, space="PSUM") as ypool,
            tc.tile_pool(name="ysb", bufs=1) as ysbp,
        ):
            # --- weights ---
            Wh_s = wpool.tile([128, KT * G], hdt, tag="Wh_s")
            nc.sync.dma_start(Wh_s[:], Wh_d[:])
            Wx_s = wpool.tile([I + 1, G], xdt, tag="Wx_s")
            nc.sync.dma_start(Wx_s[:], Wx_d[:])
            Who_s = wpool.tile([128, KT * O], fp32, tag="Who_s")
            nc.sync.dma_start(Who_s[:], Who_d[:])

            # --- state (per chain) ---
            # X_j = [2c (2 k-tiles) | tanh(gates) (8 m-slots: chat,f,i,o)]
            h_t = []
            x_t = []
            for j in range(C_):
                hj = state.tile([128, KT, Bc_], hdt, tag=f"h{j}")
                xj = state.tile([128, 2 + 8, Bc_],
                                mybir.dt.bfloat16 if XBF else fp32,
                                tag=f"X{j}")
                nc.vector.memset(hj[:], 0.0)
                nc.vector.memset(xj[:], 0.0)
                h_t.append(hj)
                x_t.append(xj)

            # --- psum ring: one tile (= one pending-zero domain) per slot ---
            ring = [
                ringp.tile([128, MT, B], fp32, tag=f"ring{r}", name=f"ring{r}")
                for r in range(R_)
            ]

            # --- x superchunk tiles, DMA'd ahead ---
            n_chunk = (t_steps + SC - 1) // SC
            xch = {}

            def fetch_chunk(ci):
                if ci in xch or ci >= n_chunk:
                    return
                cols = min(SC, t_steps - ci * SC) * B
                xt = xbuf.tile([I + 1, SC * B], xdt, tag="xch")
                nc.sync.dma_start(
                    xt[:, 0:cols], xT_d[:, ci * SC * B : ci * SC * B + cols]
                )
                xch[ci] = xt

            fetch_chunk(0)
            fetch_chunk(1)

            # --- main ladder ---
            for p in range(n_pass):
                # prefetch next x superchunk
                if p % SC == 0 and p < t_steps:
                    fetch_chunk(p // SC + 1)

                # x-phase: bulk x-projection (+bias) for steps [p, p+S)
                if p % S == 0 and p < t_steps:
                    for t in range(p, min(p + S, t_steps)):
                        slot = ring[t % R_]
                        xt = xch[t // SC]
                        rhs = xt[:, (t % SC) * B : (t % SC) * B + B]
                        for m in range(MT):
                            _tag(nc.tensor.matmul(
                                slot[:, m, :],
                                Wx_s[:, m * 128 : (m + 1) * 128],
                                rhs,
                                start=(m == 0),
                                stop=False,
                                skip_group_check=True,
                            ), "xmm", t, -1)

                # serial h-phase: all active chains, shared weight tiles
                active = [j for j in range(C_) if 0 <= p - j < t_steps]
                if M_OUTER:
                    km_order = [(k, m) for m in range(MT) for k in range(KT)]
                else:
                    km_order = [(k, m) for k in range(KT) for m in range(MT)]
                last_km = km_order[-1]
                for k, m in km_order:
                    lhsT = Wh_s[:, k * G + m * 128 : k * G + (m + 1) * 128]
                    for j in active:
                        slot = ring[(p - j) % R_]
                        _tag(nc.tensor.matmul(
                            slot[:, m, j * Bc_ : (j + 1) * Bc_],
                            lhsT,
                            h_t[j][:, k, :],
                            start=False,
                            stop=(j == C_ - 1 and (k, m) == last_km),
                            skip_group_check=True,
                        ), "hmm", p, j)

                # nonlinear phase per active chain:
                #   one tanh (all 8 gate tiles) -> 2 fused DVE cell-update
                #   ops -> tanh(c) -> h update.  c is stored as 2c inside X
                #   so no separate halving op is needed.
                for j in active:
                    slot = ring[(p - j) % R_]
                    X = x_t[j]
                    # one tanh over all 8 gate tiles into X[2:10]; slot order
                    # [chat, f, i, o] with tf/ti/to = tanh(g/2) = 2*sig(g)-1.
                    _tag(nc.scalar.activation(
                        X[:, 2:10, :], slot[:, :, j * Bc_ : (j + 1) * Bc_], Af.Tanh
                    ), "g8", p, j)
                    nld = mybir.dt.bfloat16 if XBF else fp32
                    u = tbuf.tile([128, 4, Bc_], nld, tag="u")
                    th = tbuf.tile([128, KT, Bc_], nld, tag="th")
                    mlt = mybir.AluOpType.mult
                    addo = mybir.AluOpType.add
                    # u[0:2] = (tf+1)*2c = 4fc ; u[2:4] = (ti+1)*chat = 2*i*chat
                    _tag(nc.vector.scalar_tensor_tensor(
                        u[:], X[:, 4:8, :], 1.0, X[:, 0:4, :], addo, mlt
                    ), "u1", p, j)
                    # 2c_new = 0.5*u[0:2] + u[2:4], stored back as the 2c state
                    _tag(nc.vector.scalar_tensor_tensor(
                        X[:, 0:2, :], u[:, 0:2, :], 0.5, u[:, 2:4, :], mlt, addo
                    ), "v2", p, j)
                    # th = tanh(c_new) via free input scale
                    _tag(nc.scalar.activation(
                        th[:], X[:, 0:2, :], Af.Tanh, scale=0.5), "th", p, j)
                    # h2 = (to+1)*th = 2*o*th = 2*h  (Wh, W_ho pre-halved)
                    _tag(nc.vector.scalar_tensor_tensor(
                        h_t[j][:], X[:, 8:10, :], 1.0, th[:], addo, mlt
                    ), "h2", p, j)

            # --- output projection: y = h_T @ W_ho (bias on host) ---
            # cast h to fp32 so the final projection is full precision
            # (W_ho stays fp32); reuse ring slot j's bank as the y psum.
            for j in range(C_):
                hc = ysbp.tile([128, KT, Bc_], fp32, tag=f"hc{j}", name=f"hc{j}")
                nc.vector.tensor_copy(hc[:], h_t[j][:])
                yp = ring[j][0:Bc_, 0, 0:O]
                for k in range(KT):
                    nc.tensor.matmul(
                        yp[:],
                        hc[:, k, :],
                        Who_s[:, k * O : (k + 1) * O],
                        start=(k == 0),
                        stop=(k == KT - 1),
                        skip_group_check=True,
                    )
                ys = ysbp.tile([Bc_, O], fp32, tag=f"ys{j}")
                nc.vector.tensor_copy(ys[:], yp[:])
                nc.sync.dma_start(y_d[j * Bc_ : (j + 1) * Bc_, :], ys[:])

    n = legalize_waits(nc, limit=1)
    return nc


def build_nc_variant(name):
    """Named variants for bench.py A/B runs."""
    if name == "cur":
        return build_nc()
    if name == "c1":
        return build_nc(c=1)
    raise ValueError(f"unknown variant {name}")


# ----------------------------------------------------------------------------
# Host-side packing
# ----------------------------------------------------------------------------
def _np_dt(dt):
    import ml_dtypes

    return np.float32 if dt == mybir.dt.float32 else ml_dtypes.bfloat16


def pack_weights(W_f, b_f, W_i, b_i, W_c, b_c, W_o, b_o, W_ho, hdt=None, xdt=None, t_steps=T):
    """Build Wh [128, KT*G], Wx [I+1, G], Who [128, KT*O] in packed layout."""
    np_h = _np_dt(HDT if hdt is None else hdt)
    np_x = _np_dt(XDT if xdt is None else xdt)
    Wg = np.concatenate([W_f, W_i, W_c, W_o], axis=1).astype(np.float32)  # [I+H, 4H]
    bg = np.concatenate([b_f, b_i, b_c, b_o], axis=0).astype(np.float32)  # [4H]
    # column m-tile permutation
    cols = np.concatenate(
        [np.arange(m * 128, (m + 1) * 128) for m in M_PERM]
    )
    Wg_p = Wg[:, cols]
    bg_p = bg[cols]
    # h-part rows 0:H (combined = [h, x]); x-part rows H:H+I
    Wh = Wg_p[0:H, :]                       # [256, 1024]
    Wx = Wg_p[H : H + I, :]                 # [64, 1024]
    Who = W_ho.astype(np.float32)           # [256, 8]
    if TANH_TRICK:
        # sigmoid(g) = (tanh(g/2)+1)/2: halve f,i,o gate columns (slots
        # 2:8 of the m-tile permutation; chat at slots 0:2 stays) incl.
        # bias; h is stored as 2h so all Wh rows and W_ho are halved too.
        colscale = np.ones((G,), np.float32)
        colscale[2 * 128 : 8 * 128] = 0.5
        Wh = Wh * colscale[None, :] * 0.5
        Wx = Wx * colscale[None, :]
        bg_p = bg_p * colscale
        Who = Who * 0.5
    Wx_aug = np.concatenate([Wx, bg_p[None, :]], axis=0)  # [65, 1024]
    # k-tiles side by side: [128, KT*G]
    Wh_pk = np.concatenate([Wh[k * 128 : (k + 1) * 128, :] for k in range(KT)], axis=1)
    Who_pk = np.concatenate(
        [Who[k * 128 : (k + 1) * 128, :] for k in range(KT)], axis=1
    )  # [128, 16]
    return Wh_pk.astype(np_h), Wx_aug.astype(np_x), Who_pk.astype(np.float32)


def pack_x(x, xdt=None, t_steps=T):
    """x [B_FULL, T, I] -> list of per-core xT [I+1, T*B] (with ones row)."""
    npdt = _np_dt(XDT if xdt is None else xdt)
    outs = []
    for c in range(NCORES):
        xs = np.asarray(x[c * B : (c + 1) * B, :t_steps, :], dtype=np.float32)
        xt = np.ascontiguousarray(xs.transpose(2, 1, 0))  # [I, T, B]
        ones = np.ones((1, t_steps, B), np.float32)
        xa = np.concatenate([xt, ones], axis=0).reshape(I + 1, t_steps * B)
        outs.append(xa.astype(npdt))
    return outs


# ----------------------------------------------------------------------------
# Public entry point
# ----------------------------------------------------------------------------
_CACHE = {}


def _get_nc(t_steps=T):
    key = (t_steps, str(HDT), str(XDT))
    if key not in _CACHE:
        _CACHE[key] = build_nc(t_steps)
    return _CACHE[key]


def kernel(x, W_f, b_f, W_i, b_i, W_c, b_c, W_o, b_o, W_ho, b_ho):
    from concourse.bass_utils import run_bass_kernel_spmd

    x = np.asarray(x)
    nc = _get_nc()
    Wh_pk, Wx_aug, Who_pk = pack_weights(
        W_f, b_f, W_i, b_i, W_c, b_c, W_o, b_o, W_ho
    )
    xs = pack_x(x)
    in_maps = [
        {"xT": xs[c], "Wh": Wh_pk, "Wx": Wx_aug, "Who": Who_pk}
        for c in range(NCORES)
    ]
    res = run_bass_kernel_spmd(nc, in_maps, list(range(NCORES)))
    y = np.concatenate([res.results[c]["y"] for c in range(NCORES)], axis=0)
    return (y + np.asarray(b_ho, np.float32)[None, :]).astype(np.float32)



# revision 29
# speedup vs baseline: 1.0494x; 1.0494x over previous
"""Trainium2 Bass kernel for nn_CustomLSTM (B=256, T=1024, I=64, H=256, O=8).

Strategy: data-parallel over batch across 8 cores (32 batch rows each).
Per core, the LSTM recurrence runs with everything in feature-on-partition
("transposed") layout:
  - gate pre-activations accumulate in a PSUM ring, one slot per timestep;
    x-projections (+ bias, via an appended ones-row on x) are matmul'd in
    ahead of time so the serial path only streams the Wh tiles
  - C=2 batch-chains run offset by one timestep in a ladder; each chain's
    per-step serial path is PE(16 mm) -> ACT(one tanh over all 8 gate
    tiles, via sigmoid(g)=(tanh(g/2)+1)/2 with the 1/2's folded into the
    weights) -> DVE(2 fused scalar_tensor_tensor ops) -> ACT(tanh c) ->
    DVE(h update) -> PE
  - the state tile X = [2c | tanh(gates)] with gate slot order
    [chat,f,i,o] makes the cell update 2c' = (tf+1)*2c*0.5 + (ti+1)*chat
    two DVE ops: u = (X[4:8]+1)*X[0:4]; X[0:2] = 0.5*u[0:2] + u[2:4].
    Storing c as 2c removes the c-halving op from the loop entirely.
Output projection h_T @ W_ho runs on-device; b_ho is added on the host.

This file is self-contained: shapes/sharding are hardcoded to the problem.
"""

import sys

sys.path.insert(0, "/opt/trn_rl_repo")

import numpy as np

import concourse.bass as bass
import concourse.mybir as mybir
from concourse.tile import TileContext
from concourse.vector_clock import ScopedClock, VectorClock

# ----------------------------------------------------------------------------
# Problem constants (full problem, then per-core)
# ----------------------------------------------------------------------------
B_FULL, T, I, H, O = 256, 1024, 64, 256, 8
NCORES = 8
B = B_FULL // NCORES          # 32 batch rows per core
G = 4 * H                     # 1024 gate pre-activations
KT = H // 128                 # 2 k-tiles for the h-part
MT = G // 128                 # 8 m-tiles of gate columns

# Tunables
C = 2  # batch chains (ladder depth)
Bc = B // C                   # batch per chain
# S=1: one step's x-projection per pass (keeps PE bursts small and off
# the serial h-path).
S = 1                         # x-projection lookahead (steps per x-phase)
R_EXTRA = 0                   # extra ring slots beyond the minimum (A/B knob)
R = S + 2                     # PSUM ring slots; R + C <= 8
SC = 64                       # x DMA superchunk (steps per DMA)
import os as _os
# KERNEL_DT: "bf16" (default; both paths bf16, W_ho/psum/c stay fp32),
#            "mixed" (x path fp32), "fp32" (both)
_mode = _os.environ.get("KERNEL_DT", "bf16")
HDT = mybir.dt.float32 if _mode == "fp32" else mybir.dt.bfloat16
XDT = mybir.dt.bfloat16 if _mode == "bf16" else mybir.dt.float32
SIG_SPLIT = False             # True: sig(f,i) + sig(o) separate; False: one sig op
M_OUTER = False               # True: loop m outer / k inner in the h-phase
# TANH_TRICK: sigmoid(g)=(tanh(g/2)+1)/2 with the 1/2's pre-folded into the
# weights: ONE tanh ACT op covers all 8 gate tiles. h-state is stored as
# 2h (Wh, W_ho pre-halved); c kept true via an off-critical-path halving.
TANH_TRICK = True
# XBF: keep the nonlinear-phase state tile X (2c + tanh outputs), u and th
# in bf16 -- enables DVE 2x perf mode; c-recurrence is contractive so the
# bf16 c-state costs ~2e-3 rel err over 1024 steps (verified vs numpy).
XBF = _os.environ.get("KERNEL_XBF", "0") == "1"


# m-tile permutation of gate columns: packed slot order [c0 c1 f0 f1 i0 i1 o0 o1]
# chosen so the state tile X = [2c(2) | tanh-out(8)] gives ADJACENT operands:
#   u = (X[4:8]{f,i} + 1) * X[0:4]{2c, chat}   -- one stt covers u1|u2
# reference gate column order is [f(0:256) i(256:512) c(512:768) o(768:1024)]
M_PERM = [4, 5, 0, 1, 2, 3, 6, 7]  # source m-tile index for each packed slot


# ----------------------------------------------------------------------------
# Tile walrus workaround: this container's walrus accepts at most ONE sync
# wait per instruction.  (a) patch the TileContext tail drain to spread its
# waits over per-proc SP nops; (b) after build, hoist excess waits from any
# instruction onto same-engine nops placed immediately before it.
# ----------------------------------------------------------------------------
def _patched_drain_and_barrier(self, tick_clock, wait_clock):
    nc = self.nc
    g = tick_clock.global_clock
    n = len(g)
    for p in range(n):
        if g[p] == 0:
            continue
        vc = VectorClock([g[q] if q == p else 0 for q in range(n)])
        nop = nc.sync.nop(nofuse=True)
        wait_clock.add_sem_waits(nop.ins, ScopedClock({None: vc}))
    nc.sync.drain()
    nc.all_engine_barrier()
    assert self.sems is not None
    popped = nc._tile_sem_poison_stack.pop()
    assert popped is self._sem_poison
    nc.clear_and_free_semaphores(list(self.sems.allocated().values()))
    nc.all_engine_barrier()


def apply_tile_patch():
    TileContext._drain_and_barrier = _patched_drain_and_barrier


def legalize_waits(nc, limit=1):
    """Hoist excess sem waits (>limit per instruction) onto same-engine nops
    inserted immediately before the instruction."""
    eng_builders = {
        mybir.EngineType.PE: nc.tensor,
        mybir.EngineType.DVE: nc.vector,
        mybir.EngineType.Activation: nc.scalar,
        mybir.EngineType.Pool: nc.gpsimd,
        mybir.EngineType.SP: nc.sync,
    }
    n_hoisted = 0
    for f in nc.m.functions:
        for bb in f.blocks:
            snapshot = list(bb.instructions)
            fixes = []  # (index, inst, excess_waits)
            for idx, inst in enumerate(snapshot):
                si = inst.sync_info
                waits = list(si.on_wait) if si and si.on_wait else []
                if len(waits) > limit:
                    fixes.append((idx, inst, waits))
            if not fixes:
                continue
            # create nops via the engine builders (they append to cur_bb;
            # pop them back off to place manually)
            out = []
            prev = 0
            for idx, inst, waits in fixes:
                out.extend(snapshot[prev:idx])
                keep = waits[-limit:]
                excess = waits[:-limit]
                for w in excess:
                    builder = eng_builders[inst.engine]
                    nop_bi = builder.nop(nofuse=True)
                    nop_inst = nop_bi.ins
                    # remove from wherever the builder appended it
                    cur = nc.cur_bb.bb
                    assert cur.instructions[-1] is nop_inst
                    cur.instructions.pop()
                    nop_inst.sync_info = mybir.SyncInfo(on_wait=[w], on_update=[])
                    out.append(nop_inst)
                    n_hoisted += 1
                inst.sync_info = mybir.SyncInfo(
                    on_wait=keep, on_update=list(inst.sync_info.on_update or [])
                )
                out.append(inst)
                prev = idx + 1
            out.extend(snapshot[prev:])
            bb.instructions = out
    return n_hoisted


# ----------------------------------------------------------------------------
# Kernel build
# ----------------------------------------------------------------------------
ROLES = {}  # inst name -> (role, pass, chain); populated during build for sim


def _tag(bi, role, p, j):
    try:
        ROLES[bi.ins.name] = (role, p, j)
    except Exception:
        pass
    return bi


def build_nc(t_steps=T, hdt=None, xdt=None, dt=None, c=None):
    """Build the per-core Bass program. Returns nc."""
    ROLES.clear()
    C_ = C if c is None else c
    Bc_ = B // C_
    R_ = S + C_ + R_EXTRA
    if dt is not None:
        hdt = xdt = dt
    hdt = HDT if hdt is None else hdt
    xdt = XDT if xdt is None else xdt
    apply_tile_patch()
    fp32 = mybir.dt.float32
    Af = mybir.ActivationFunctionType

    nc = bass.Bass()
    xT_d = nc.dram_tensor("xT", [I + 1, t_steps * B], xdt, kind="ExternalInput")
    Wh_d = nc.dram_tensor("Wh", [128, KT * G], hdt, kind="ExternalInput")
    Wx_d = nc.dram_tensor("Wx", [I + 1, G], xdt, kind="ExternalInput")
    Who_d = nc.dram_tensor("Who", [128, KT * O], fp32, kind="ExternalInput")
    y_d = nc.dram_tensor("y", [B, O], fp32, kind="ExternalOutput")

    n_pass = t_steps + C_ - 1

    with TileContext(nc) as tc:
        with (
            tc.tile_pool(name="wpool", bufs=1) as wpool,
            tc.tile_pool(name="state", bufs=1) as state,
            tc.tile_pool(name="xbuf", bufs=3) as xbuf,
            tc.tile_pool(name="gbuf", bufs=2 * C_ + 1) as gbuf,
            tc.tile_pool(name="tbuf", bufs=2 * C_ + 2) as tbuf,
            tc.tile_pool(name="ring", bufs=1, space="PSUM") as ringp,
            tc.tile_pool(name="ypsum", bufs=1, space="PSUM") as ypool,
            tc.tile_pool(name="ysb", bufs=1) as ysbp,
        ):
            # --- weights ---
            Wh_s = wpool.tile([128, KT * G], hdt, tag="Wh_s")
            nc.sync.dma_start(Wh_s[:], Wh_d[:])
            Wx_s = wpool.tile([I + 1, G], xdt, tag="Wx_s")
            nc.sync.dma_start(Wx_s[:], Wx_d[:])
            Who_s = wpool.tile([128, KT * O], fp32, tag="Who_s")
            nc.sync.dma_start(Who_s[:], Who_d[:])

            # --- state (per chain) ---
            # X_j = [2c (2 k-tiles) | tanh(gates) (8 m-slots: chat,f,i,o)]
            h_t = []
            x_t = []
            for j in range(C_):
                hj = state.tile([128, KT, Bc_], hdt, tag=f"h{j}")
                xj = state.tile([128, 2 + 8, Bc_],
                                mybir.dt.bfloat16 if XBF else fp32,
                                tag=f"X{j}")
                nc.vector.memset(hj[:], 0.0)
                nc.vector.memset(xj[:], 0.0)
                h_t.append(hj)
                x_t.append(xj)

            # --- psum ring: one tile (= one pending-zero domain) per slot ---
            ring = [
                ringp.tile([128, MT, B], fp32, tag=f"ring{r}", name=f"ring{r}")
                for r in range(R_)
            ]

            # --- x superchunk tiles, DMA'd ahead ---
            n_chunk = (t_steps + SC - 1) // SC
            xch = {}

            def fetch_chunk(ci):
                if ci in xch or ci >= n_chunk:
                    return
                cols = min(SC, t_steps - ci * SC) * B
                xt = xbuf.tile([I + 1, SC * B], xdt, tag="xch")
                nc.sync.dma_start(
                    xt[:, 0:cols], xT_d[:, ci * SC * B : ci * SC * B + cols]
                )
                xch[ci] = xt

            fetch_chunk(0)
            fetch_chunk(1)

            # --- main ladder ---
            for p in range(n_pass):
                # prefetch next x superchunk
                if p % SC == 0 and p < t_steps:
                    fetch_chunk(p // SC + 1)

                # x-phase: bulk x-projection (+bias) for steps [p, p+S)
                if p % S == 0 and p < t_steps:
                    for t in range(p, min(p + S, t_steps)):
                        slot = ring[t % R_]
                        xt = xch[t // SC]
                        rhs = xt[:, (t % SC) * B : (t % SC) * B + B]
                        for m in range(MT):
                            _tag(nc.tensor.matmul(
                                slot[:, m, :],
                                Wx_s[:, m * 128 : (m + 1) * 128],
                                rhs,
                                start=(m == 0),
                                stop=False,
                                skip_group_check=True,
                            ), "xmm", t, -1)

                # serial h-phase: all active chains, shared weight tiles
                active = [j for j in range(C_) if 0 <= p - j < t_steps]
                if M_OUTER:
                    km_order = [(k, m) for m in range(MT) for k in range(KT)]
                else:
                    km_order = [(k, m) for k in range(KT) for m in range(MT)]
                last_km = km_order[-1]
                for k, m in km_order:
                    lhsT = Wh_s[:, k * G + m * 128 : k * G + (m + 1) * 128]
                    for j in active:
                        slot = ring[(p - j) % R_]
                        _tag(nc.tensor.matmul(
                            slot[:, m, j * Bc_ : (j + 1) * Bc_],
                            lhsT,
                            h_t[j][:, k, :],
                            start=False,
                            stop=(j == C_ - 1 and (k, m) == last_km),
                            skip_group_check=True,
                        ), "hmm", p, j)

                # nonlinear phase per active chain:
                #   one tanh (all 8 gate tiles) -> 2 fused DVE cell-update
                #   ops -> tanh(c) -> h update.  c is stored as 2c inside X
                #   so no separate halving op is needed.
                for j in active:
                    slot = ring[(p - j) % R_]
                    X = x_t[j]
                    # one tanh over all 8 gate tiles into X[2:10]; slot order
                    # [chat, f, i, o] with tf/ti/to = tanh(g/2) = 2*sig(g)-1.
                    _tag(nc.scalar.activation(
                        X[:, 2:10, :], slot[:, :, j * Bc_ : (j + 1) * Bc_], Af.Tanh
                    ), "g8", p, j)
                    nld = mybir.dt.bfloat16 if XBF else fp32
                    u = tbuf.tile([128, 4, Bc_], nld, tag="u")
                    th = tbuf.tile([128, KT, Bc_], nld, tag="th")
                    mlt = mybir.AluOpType.mult
                    addo = mybir.AluOpType.add
                    # u[0:2] = (tf+1)*2c = 4fc ; u[2:4] = (ti+1)*chat = 2*i*chat
                    _tag(nc.vector.scalar_tensor_tensor(
                        u[:], X[:, 4:8, :], 1.0, X[:, 0:4, :], addo, mlt
                    ), "u1", p, j)
                    # 2c_new = 0.5*u[0:2] + u[2:4], stored back as the 2c state
                    _tag(nc.vector.scalar_tensor_tensor(
                        X[:, 0:2, :], u[:, 0:2, :], 0.5, u[:, 2:4, :], mlt, addo
                    ), "v2", p, j)
                    # th = tanh(c_new) via free input scale
                    _tag(nc.scalar.activation(
                        th[:], X[:, 0:2, :], Af.Tanh, scale=0.5), "th", p, j)
                    # h2 = (to+1)*th = 2*o*th = 2*h  (Wh, W_ho pre-halved)
                    _tag(nc.vector.scalar_tensor_tensor(
                        h_t[j][:], X[:, 8:10, :], 1.0, th[:], addo, mlt
                    ), "h2", p, j)

            # --- output projection: y = h_T @ W_ho (bias on host) ---
            # cast h to fp32 so the final projection is full precision
            # (W_ho stays fp32); reuse ring slot j's bank as the y psum.
            for j in range(C_):
                hc = ysbp.tile([128, KT, Bc_], fp32, tag=f"hc{j}", name=f"hc{j}")
                nc.vector.tensor_copy(hc[:], h_t[j][:])
                yp = ring[j][0:Bc_, 0, 0:O]
                for k in range(KT):
                    nc.tensor.matmul(
                        yp[:],
                        hc[:, k, :],
                        Who_s[:, k * O : (k + 1) * O],
                        start=(k == 0),
                        stop=(k == KT - 1),
                        skip_group_check=True,
                    )
                ys = ysbp.tile([Bc_, O], fp32, tag=f"ys{j}")
                nc.vector.tensor_copy(ys[:], yp[:])
                nc.sync.dma_start(y_d[j * Bc_ : (j + 1) * Bc_, :], ys[:])

    n = legalize_waits(nc, limit=1)
    return nc


def build_nc_variant(name):
    """Named variants for bench.py A/B runs."""
    global R_EXTRA, M_OUTER
    if name == "cur":
        return build_nc()
    if name == "c1":
        return build_nc(c=1)
    if name == "r4":
        R_EXTRA = 1
        try:
            return build_nc()
        finally:
            R_EXTRA = 0
    if name == "mo":
        M_OUTER = True
        try:
            return build_nc()
        finally:
            M_OUTER = False
    raise ValueError(f"unknown variant {name}")


# ----------------------------------------------------------------------------
# Host-side packing
# ----------------------------------------------------------------------------
def _np_dt(dt):
    import ml_dtypes

    return np.float32 if dt == mybir.dt.float32 else ml_dtypes.bfloat16


def pack_weights(W_f, b_f, W_i, b_i, W_c, b_c, W_o, b_o, W_ho, hdt=None, xdt=None, t_steps=T):
    """Build Wh [128, KT*G], Wx [I+1, G], Who [128, KT*O] in packed layout."""
    np_h = _np_dt(HDT if hdt is None else hdt)
    np_x = _np_dt(XDT if xdt is None else xdt)
    Wg = np.concatenate([W_f, W_i, W_c, W_o], axis=1).astype(np.float32)  # [I+H, 4H]
    bg = np.concatenate([b_f, b_i, b_c, b_o], axis=0).astype(np.float32)  # [4H]
    # column m-tile permutation
    cols = np.concatenate(
        [np.arange(m * 128, (m + 1) * 128) for m in M_PERM]
    )
    Wg_p = Wg[:, cols]
    bg_p = bg[cols]
    # h-part rows 0:H (combined = [h, x]); x-part rows H:H+I
    Wh = Wg_p[0:H, :]                       # [256, 1024]
    Wx = Wg_p[H : H + I, :]                 # [64, 1024]
    Who = W_ho.astype(np.float32)           # [256, 8]
    if TANH_TRICK:
        # sigmoid(g) = (tanh(g/2)+1)/2: halve f,i,o gate columns (slots
        # 2:8 of the m-tile permutation; chat at slots 0:2 stays) incl.
        # bias; h is stored as 2h so all Wh rows and W_ho are halved too.
        colscale = np.ones((G,), np.float32)
        colscale[2 * 128 : 8 * 128] = 0.5
        Wh = Wh * colscale[None, :] * 0.5
        Wx = Wx * colscale[None, :]
        bg_p = bg_p * colscale
        Who = Who * 0.5
    Wx_aug = np.concatenate([Wx, bg_p[None, :]], axis=0)  # [65, 1024]
    # k-tiles side by side: [128, KT*G]
    Wh_pk = np.concatenate([Wh[k * 128 : (k + 1) * 128, :] for k in range(KT)], axis=1)
    Who_pk = np.concatenate(
        [Who[k * 128 : (k + 1) * 128, :] for k in range(KT)], axis=1
    )  # [128, 16]
    return Wh_pk.astype(np_h), Wx_aug.astype(np_x), Who_pk.astype(np.float32)


def pack_x(x, xdt=None, t_steps=T):
    """x [B_FULL, T, I] -> list of per-core xT [I+1, T*B] (with ones row)."""
    npdt = _np_dt(XDT if xdt is None else xdt)
    outs = []
    for c in range(NCORES):
        xs = np.asarray(x[c * B : (c + 1) * B, :t_steps, :], dtype=np.float32)
        xt = np.ascontiguousarray(xs.transpose(2, 1, 0))  # [I, T, B]
        ones = np.ones((1, t_steps, B), np.float32)
        xa = np.concatenate([xt, ones], axis=0).reshape(I + 1, t_steps * B)
        outs.append(xa.astype(npdt))
    return outs


# ----------------------------------------------------------------------------
# Public entry point
# ----------------------------------------------------------------------------
_CACHE = {}


def _get_nc(t_steps=T):
    key = (t_steps, str(HDT), str(XDT))
    if key not in _CACHE:
        _CACHE[key] = build_nc(t_steps)
    return _CACHE[key]


def kernel(x, W_f, b_f, W_i, b_i, W_c, b_c, W_o, b_o, W_ho, b_ho):
    from concourse.bass_utils import run_bass_kernel_spmd

    x = np.asarray(x)
    nc = _get_nc()
    Wh_pk, Wx_aug, Who_pk = pack_weights(
        W_f, b_f, W_i, b_i, W_c, b_c, W_o, b_o, W_ho
    )
    xs = pack_x(x)
    in_maps = [
        {"xT": xs[c], "Wh": Wh_pk, "Wx": Wx_aug, "Who": Who_pk}
        for c in range(NCORES)
    ]
    res = run_bass_kernel_spmd(nc, in_maps, list(range(NCORES)))
    y = np.concatenate([res.results[c]["y"] for c in range(NCORES)], axis=0)
    return (y + np.asarray(b_ho, np.float32)[None, :]).astype(np.float32)



# revision 33
# speedup vs baseline: 1.3974x; 1.3315x over previous
"""Trainium2 Bass kernel for nn_CustomLSTM (B=256, T=1024, I=64, H=256, O=8).

Strategy: data-parallel over batch across 8 cores (32 batch rows each).
Per core, the LSTM recurrence runs with everything in feature-on-partition
("transposed") layout:
  - gate pre-activations accumulate in a PSUM ring, one slot per timestep;
    x-projections (+ bias, via an appended ones-row on x) are matmul'd in
    ahead of time so the serial path only streams the Wh tiles
  - C=2 batch-chains run offset by one timestep in a ladder; each chain's
    per-step serial path is PE(16 mm) -> ACT(one tanh over all 8 gate
    tiles, via sigmoid(g)=(tanh(g/2)+1)/2 with the 1/2's folded into the
    weights) -> DVE(2 fused scalar_tensor_tensor ops) -> ACT(tanh c) ->
    DVE(h update) -> PE
  - the state tile X = [2c | tanh(gates)] with gate slot order
    [chat,f,i,o] makes the cell update 2c' = (tf+1)*2c*0.5 + (ti+1)*chat
    two DVE ops: u = (X[4:8]+1)*X[0:4]; X[0:2] = 0.5*u[0:2] + u[2:4].
    Storing c as 2c removes the c-halving op from the loop entirely.
Output projection h_T @ W_ho runs on-device; b_ho is added on the host.

This file is self-contained: shapes/sharding are hardcoded to the problem.
"""

import sys

sys.path.insert(0, "/opt/trn_rl_repo")

import numpy as np

import concourse.bass as bass
import concourse.mybir as mybir
from concourse.tile import TileContext
from concourse.vector_clock import ScopedClock, VectorClock

# ----------------------------------------------------------------------------
# Problem constants (full problem, then per-core)
# ----------------------------------------------------------------------------
B_FULL, T, I, H, O = 256, 1024, 64, 256, 8
NCORES = 8
B = B_FULL // NCORES          # 32 batch rows per core
G = 4 * H                     # 1024 gate pre-activations
KT = H // 128                 # 2 k-tiles for the h-part
MT = G // 128                 # 8 m-tiles of gate columns

# Tunables
C = 2  # batch chains (ladder depth)
Bc = B // C                   # batch per chain
# S=1: one step's x-projection per pass (keeps PE bursts small and off
# the serial h-path).
S = 1                         # x-projection lookahead (steps per x-phase)
R_EXTRA = 0                   # extra ring slots beyond the minimum (A/B knob)
R = S + 2                     # PSUM ring slots; R + C <= 8
SC = 64                       # x DMA superchunk (steps per DMA)
import os as _os
# KERNEL_DT: "bf16" (default; both paths bf16, W_ho/psum/c stay fp32),
#            "mixed" (x path fp32), "fp32" (both)
_mode = _os.environ.get("KERNEL_DT", "bf16")
HDT = mybir.dt.float32 if _mode == "fp32" else mybir.dt.bfloat16
XDT = mybir.dt.bfloat16 if _mode == "bf16" else mybir.dt.float32
SIG_SPLIT = False             # True: sig(f,i) + sig(o) separate; False: one sig op
M_OUTER = False               # True: loop m outer / k inner in the h-phase
# TANH_TRICK: sigmoid(g)=(tanh(g/2)+1)/2 with the 1/2's pre-folded into the
# weights: ONE tanh ACT op covers all 8 gate tiles. h-state is stored as
# 2h (Wh, W_ho pre-halved); c kept true via an off-critical-path halving.
TANH_TRICK = True
# XBF: keep the nonlinear-phase state tile X (2c + tanh outputs), u and th
# in bf16 -- enables DVE 2x perf mode; c-recurrence is contractive so the
# bf16 c-state costs ~2e-3 rel err over 1024 steps (verified vs numpy).
XBF = _os.environ.get("KERNEL_XBF", "0") == "1"


# m-tile permutation of gate columns: packed slot order [c0 c1 f0 f1 i0 i1 o0 o1]
# chosen so the state tile X = [2c(2) | tanh-out(8)] gives ADJACENT operands:
#   u = (X[4:8]{f,i} + 1) * X[0:4]{2c, chat}   -- one stt covers u1|u2
# reference gate column order is [f(0:256) i(256:512) c(512:768) o(768:1024)]
M_PERM = [4, 5, 0, 1, 2, 3, 6, 7]  # source m-tile index for each packed slot


# ----------------------------------------------------------------------------
# Tile walrus workaround: this container's walrus accepts at most ONE sync
# wait per instruction.  (a) patch the TileContext tail drain to spread its
# waits over per-proc SP nops; (b) after build, hoist excess waits from any
# instruction onto same-engine nops placed immediately before it.
# ----------------------------------------------------------------------------
def _patched_drain_and_barrier(self, tick_clock, wait_clock):
    nc = self.nc
    g = tick_clock.global_clock
    n = len(g)
    for p in range(n):
        if g[p] == 0:
            continue
        vc = VectorClock([g[q] if q == p else 0 for q in range(n)])
        nop = nc.sync.nop(nofuse=True)
        wait_clock.add_sem_waits(nop.ins, ScopedClock({None: vc}))
    nc.sync.drain()
    nc.all_engine_barrier()
    assert self.sems is not None
    popped = nc._tile_sem_poison_stack.pop()
    assert popped is self._sem_poison
    nc.clear_and_free_semaphores(list(self.sems.allocated().values()))
    nc.all_engine_barrier()


def apply_tile_patch():
    TileContext._drain_and_barrier = _patched_drain_and_barrier


def legalize_waits(nc, limit=1):
    """Hoist excess sem waits (>limit per instruction) onto same-engine nops
    inserted immediately before the instruction."""
    eng_builders = {
        mybir.EngineType.PE: nc.tensor,
        mybir.EngineType.DVE: nc.vector,
        mybir.EngineType.Activation: nc.scalar,
        mybir.EngineType.Pool: nc.gpsimd,
        mybir.EngineType.SP: nc.sync,
    }
    n_hoisted = 0
    for f in nc.m.functions:
        for bb in f.blocks:
            snapshot = list(bb.instructions)
            fixes = []  # (index, inst, excess_waits)
            for idx, inst in enumerate(snapshot):
                si = inst.sync_info
                waits = list(si.on_wait) if si and si.on_wait else []
                if len(waits) > limit:
                    fixes.append((idx, inst, waits))
            if not fixes:
                continue
            # create nops via the engine builders (they append to cur_bb;
            # pop them back off to place manually)
            out = []
            prev = 0
            for idx, inst, waits in fixes:
                out.extend(snapshot[prev:idx])
                keep = waits[-limit:]
                excess = waits[:-limit]
                for w in excess:
                    builder = eng_builders[inst.engine]
                    nop_bi = builder.nop(nofuse=True)
                    nop_inst = nop_bi.ins
                    # remove from wherever the builder appended it
                    cur = nc.cur_bb.bb
                    assert cur.instructions[-1] is nop_inst
                    cur.instructions.pop()
                    nop_inst.sync_info = mybir.SyncInfo(on_wait=[w], on_update=[])
                    out.append(nop_inst)
                    n_hoisted += 1
                inst.sync_info = mybir.SyncInfo(
                    on_wait=keep, on_update=list(inst.sync_info.on_update or [])
                )
                out.append(inst)
                prev = idx + 1
            out.extend(snapshot[prev:])
            bb.instructions = out
    return n_hoisted


# ----------------------------------------------------------------------------
# Kernel build
# ----------------------------------------------------------------------------
ROLES = {}  # inst name -> (role, pass, chain); populated during build for sim


def _tag(bi, role, p, j):
    try:
        ROLES[bi.ins.name] = (role, p, j)
    except Exception:
        pass
    return bi


def build_nc(t_steps=T, hdt=None, xdt=None, dt=None, c=None):
    """Build the per-core Bass program. Returns nc."""
    ROLES.clear()
    C_ = C if c is None else c
    Bc_ = B // C_
    R_ = S + C_ + R_EXTRA
    if dt is not None:
        hdt = xdt = dt
    hdt = HDT if hdt is None else hdt
    xdt = XDT if xdt is None else xdt
    apply_tile_patch()
    fp32 = mybir.dt.float32
    Af = mybir.ActivationFunctionType

    nc = bass.Bass()
    xT_d = nc.dram_tensor("xT", [I + 1, t_steps * B], xdt, kind="ExternalInput")
    Wh_d = nc.dram_tensor("Wh", [128, KT * G], hdt, kind="ExternalInput")
    Wx_d = nc.dram_tensor("Wx", [I + 1, G], xdt, kind="ExternalInput")
    Who_d = nc.dram_tensor("Who", [128, KT * O], fp32, kind="ExternalInput")
    # y is shipped as [Bc, C, O] (chain-major free dim) in ONE DMA; the
    # host reorders to [B, O] (b = j*Bc + p  <->  y[p, j]).
    y_d = nc.dram_tensor("y", [B // C_, C_, O], fp32, kind="ExternalOutput")

    n_pass = t_steps + C_ - 1

    with TileContext(nc) as tc:
        with (
            tc.tile_pool(name="wpool", bufs=1) as wpool,
            tc.tile_pool(name="state", bufs=1) as state,
            tc.tile_pool(name="xbuf", bufs=3) as xbuf,
            tc.tile_pool(name="gbuf", bufs=2 * C_ + 1) as gbuf,
            tc.tile_pool(name="tbuf", bufs=2 * C_ + 2) as tbuf,
            tc.tile_pool(name="ring", bufs=1, space="PSUM") as ringp,
            tc.tile_pool(name="ypsum", bufs=1, space="PSUM") as ypool,
            tc.tile_pool(name="ysb", bufs=1) as ysbp,
        ):
            # --- weights: issue from different engines' DGE queues so the
            # startup DMAs run in parallel instead of serializing on SP
            # (x chunk 0 stays on SP; Who is only needed at the very end) ---
            Wh_s = wpool.tile([128, KT * G], hdt, tag="Wh_s")
            nc.scalar.dma_start(Wh_s[:], Wh_d[:])
            Wx_s = wpool.tile([I + 1, G], xdt, tag="Wx_s")
            nc.gpsimd.dma_start(Wx_s[:], Wx_d[:])
            Who_s = wpool.tile([128, KT * O], fp32, tag="Who_s")
            nc.gpsimd.dma_start(Who_s[:], Who_d[:])

            # --- state (per chain) ---
            # X_j = [2c (2 k-tiles) | tanh(gates) (8 m-slots: chat,f,i,o)]
            h_t = []
            x_t = []
            for j in range(C_):
                hj = state.tile([128, KT, Bc_], hdt, tag=f"h{j}")
                xj = state.tile([128, 2 + 8, Bc_],
                                mybir.dt.bfloat16 if XBF else fp32,
                                tag=f"X{j}")
                nc.vector.memset(hj[:], 0.0)
                nc.vector.memset(xj[:], 0.0)
                h_t.append(hj)
                x_t.append(xj)

            # --- psum ring: one tile (= one pending-zero domain) per slot ---
            ring = [
                ringp.tile([128, MT, B], fp32, tag=f"ring{r}", name=f"ring{r}")
                for r in range(R_)
            ]

            # --- x superchunk tiles, DMA'd ahead ---
            n_chunk = (t_steps + SC - 1) // SC
            xch = {}

            def fetch_chunk(ci):
                if ci in xch or ci >= n_chunk:
                    return
                cols = min(SC, t_steps - ci * SC) * B
                xt = xbuf.tile([I + 1, SC * B], xdt, tag="xch")
                nc.sync.dma_start(
                    xt[:, 0:cols], xT_d[:, ci * SC * B : ci * SC * B + cols]
                )
                xch[ci] = xt

            fetch_chunk(0)
            fetch_chunk(1)

            # --- main ladder ---
            for p in range(n_pass):
                # prefetch next x superchunk
                if p % SC == 0 and p < t_steps:
                    fetch_chunk(p // SC + 1)

                # x-phase: bulk x-projection (+bias) for steps [p, p+S)
                if p % S == 0 and p < t_steps:
                    for t in range(p, min(p + S, t_steps)):
                        slot = ring[t % R_]
                        xt = xch[t // SC]
                        rhs = xt[:, (t % SC) * B : (t % SC) * B + B]
                        for m in range(MT):
                            _tag(nc.tensor.matmul(
                                slot[:, m, :],
                                Wx_s[:, m * 128 : (m + 1) * 128],
                                rhs,
                                start=(m == 0),
                                stop=False,
                                skip_group_check=True,
                            ), "xmm", t, -1)

                # serial h-phase: all active chains, shared weight tiles
                active = [j for j in range(C_) if 0 <= p - j < t_steps]
                if M_OUTER:
                    km_order = [(k, m) for m in range(MT) for k in range(KT)]
                else:
                    km_order = [(k, m) for k in range(KT) for m in range(MT)]
                last_km = km_order[-1]
                for k, m in km_order:
                    lhsT = Wh_s[:, k * G + m * 128 : k * G + (m + 1) * 128]
                    for j in active:
                        slot = ring[(p - j) % R_]
                        _tag(nc.tensor.matmul(
                            slot[:, m, j * Bc_ : (j + 1) * Bc_],
                            lhsT,
                            h_t[j][:, k, :],
                            start=False,
                            stop=(j == C_ - 1 and (k, m) == last_km),
                            skip_group_check=True,
                        ), "hmm", p, j)

                # nonlinear phase per active chain:
                #   one tanh (all 8 gate tiles) -> 2 fused DVE cell-update
                #   ops -> tanh(c) -> h update.  c is stored as 2c inside X
                #   so no separate halving op is needed.
                for j in active:
                    slot = ring[(p - j) % R_]
                    X = x_t[j]
                    # one tanh over all 8 gate tiles into X[2:10]; slot order
                    # [chat, f, i, o] with tf/ti/to = tanh(g/2) = 2*sig(g)-1.
                    _tag(nc.scalar.activation(
                        X[:, 2:10, :], slot[:, :, j * Bc_ : (j + 1) * Bc_], Af.Tanh
                    ), "g8", p, j)
                    nld = mybir.dt.bfloat16 if XBF else fp32
                    u = tbuf.tile([128, 4, Bc_], nld, tag="u")
                    th = tbuf.tile([128, KT, Bc_], nld, tag="th")
                    mlt = mybir.AluOpType.mult
                    addo = mybir.AluOpType.add
                    # u[0:2] = (tf+1)*2c = 4fc ; u[2:4] = (ti+1)*chat = 2*i*chat
                    _tag(nc.vector.scalar_tensor_tensor(
                        u[:], X[:, 4:8, :], 1.0, X[:, 0:4, :], addo, mlt
                    ), "u1", p, j)
                    # 2c_new = 0.5*u[0:2] + u[2:4], stored back as the 2c state
                    _tag(nc.vector.scalar_tensor_tensor(
                        X[:, 0:2, :], u[:, 0:2, :], 0.5, u[:, 2:4, :], mlt, addo
                    ), "v2", p, j)
                    # th = tanh(c_new) via free input scale
                    _tag(nc.scalar.activation(
                        th[:], X[:, 0:2, :], Af.Tanh, scale=0.5), "th", p, j)
                    # h2 = (to+1)*th = 2*o*th = 2*h  (Wh, W_ho pre-halved)
                    _tag(nc.vector.scalar_tensor_tensor(
                        h_t[j][:], X[:, 8:10, :], 1.0, th[:], addo, mlt
                    ), "h2", p, j)

            # --- output projection: y = h_T @ W_ho (bias on host) ---
            # cast h to fp32 so the final projection is full precision
            # (W_ho stays fp32); reuse ring slot j's bank as the y psum.
            # All chains' results gather into ONE ys tile so the output
            # ships as a single DMA (each DMA has ~2us fixed overhead).
            ys = ysbp.tile([Bc_, C_, O], fp32, tag="ys", name="ys")
            for j in range(C_):
                hc = ysbp.tile([128, KT, Bc_], fp32, tag=f"hc{j}", name=f"hc{j}")
                nc.vector.tensor_copy(hc[:], h_t[j][:])
                yp = ring[j][0:Bc_, 0, 0:O]
                for k in range(KT):
                    nc.tensor.matmul(
                        yp[:],
                        hc[:, k, :],
                        Who_s[:, k * O : (k + 1) * O],
                        start=(k == 0),
                        stop=(k == KT - 1),
                        skip_group_check=True,
                    )
                nc.vector.tensor_copy(ys[:, j, :], yp[:])
            nc.sync.dma_start(y_d[:], ys[:])

    n = legalize_waits(nc, limit=1)
    return nc


def build_nc_variant(name):
    """Named variants for bench.py A/B runs."""
    global R_EXTRA, M_OUTER
    if name == "cur":
        return build_nc()
    if name == "c1":
        return build_nc(c=1)
    if name == "r4":
        R_EXTRA = 1
        try:
            return build_nc()
        finally:
            R_EXTRA = 0
    if name == "mo":
        M_OUTER = True
        try:
            return build_nc()
        finally:
            M_OUTER = False
    raise ValueError(f"unknown variant {name}")


# ----------------------------------------------------------------------------
# Host-side packing
# ----------------------------------------------------------------------------
def _np_dt(dt):
    import ml_dtypes

    return np.float32 if dt == mybir.dt.float32 else ml_dtypes.bfloat16


def pack_weights(W_f, b_f, W_i, b_i, W_c, b_c, W_o, b_o, W_ho, hdt=None, xdt=None, t_steps=T):
    """Build Wh [128, KT*G], Wx [I+1, G], Who [128, KT*O] in packed layout."""
    np_h = _np_dt(HDT if hdt is None else hdt)
    np_x = _np_dt(XDT if xdt is None else xdt)
    Wg = np.concatenate([W_f, W_i, W_c, W_o], axis=1).astype(np.float32)  # [I+H, 4H]
    bg = np.concatenate([b_f, b_i, b_c, b_o], axis=0).astype(np.float32)  # [4H]
    # column m-tile permutation
    cols = np.concatenate(
        [np.arange(m * 128, (m + 1) * 128) for m in M_PERM]
    )
    Wg_p = Wg[:, cols]
    bg_p = bg[cols]
    # h-part rows 0:H (combined = [h, x]); x-part rows H:H+I
    Wh = Wg_p[0:H, :]                       # [256, 1024]
    Wx = Wg_p[H : H + I, :]                 # [64, 1024]
    Who = W_ho.astype(np.float32)           # [256, 8]
    if TANH_TRICK:
        # sigmoid(g) = (tanh(g/2)+1)/2: halve f,i,o gate columns (slots
        # 2:8 of the m-tile permutation; chat at slots 0:2 stays) incl.
        # bias; h is stored as 2h so all Wh rows and W_ho are halved too.
        colscale = np.ones((G,), np.float32)
        colscale[2 * 128 : 8 * 128] = 0.5
        Wh = Wh * colscale[None, :] * 0.5
        Wx = Wx * colscale[None, :]
        bg_p = bg_p * colscale
        Who = Who * 0.5
    Wx_aug = np.concatenate([Wx, bg_p[None, :]], axis=0)  # [65, 1024]
    # k-tiles side by side: [128, KT*G]
    Wh_pk = np.concatenate([Wh[k * 128 : (k + 1) * 128, :] for k in range(KT)], axis=1)
    Who_pk = np.concatenate(
        [Who[k * 128 : (k + 1) * 128, :] for k in range(KT)], axis=1
    )  # [128, 16]
    return Wh_pk.astype(np_h), Wx_aug.astype(np_x), Who_pk.astype(np.float32)


def pack_x(x, xdt=None, t_steps=T):
    """x [B_FULL, T, I] -> list of per-core xT [I+1, T*B] (with ones row)."""
    npdt = _np_dt(XDT if xdt is None else xdt)
    outs = []
    for c in range(NCORES):
        xs = np.asarray(x[c * B : (c + 1) * B, :t_steps, :], dtype=np.float32)
        xt = np.ascontiguousarray(xs.transpose(2, 1, 0))  # [I, T, B]
        ones = np.ones((1, t_steps, B), np.float32)
        xa = np.concatenate([xt, ones], axis=0).reshape(I + 1, t_steps * B)
        outs.append(xa.astype(npdt))
    return outs


# ----------------------------------------------------------------------------
# Public entry point
# ----------------------------------------------------------------------------
_CACHE = {}


def _get_nc(t_steps=T):
    key = (t_steps, str(HDT), str(XDT))
    if key not in _CACHE:
        _CACHE[key] = build_nc(t_steps)
    return _CACHE[key]


def kernel(x, W_f, b_f, W_i, b_i, W_c, b_c, W_o, b_o, W_ho, b_ho):
    from concourse.bass_utils import run_bass_kernel_spmd

    x = np.asarray(x)
    nc = _get_nc()
    Wh_pk, Wx_aug, Who_pk = pack_weights(
        W_f, b_f, W_i, b_i, W_c, b_c, W_o, b_o, W_ho
    )
    xs = pack_x(x)
    in_maps = [
        {"xT": xs[c], "Wh": Wh_pk, "Wx": Wx_aug, "Who": Who_pk}
        for c in range(NCORES)
    ]
    res = run_bass_kernel_spmd(nc, in_maps, list(range(NCORES)))
    # per-core y arrives as [Bc, C, O]: batch row b = j*Bc + p sits at [p, j]
    y = np.concatenate(
        [np.transpose(res.results[c]["y"], (1, 0, 2)).reshape(B, O)
         for c in range(NCORES)],
        axis=0,
    )
    return (y + np.asarray(b_ho, np.float32)[None, :]).astype(np.float32)

